# revision 4
# baseline (speedup 1.0000x reference)
"""Multi-head attention (with the repo's k=q bug) on 8 Trainium2 NeuronCores.

Reference computation (B=2, S=2048, D=512, H=8, DK=64):
    q = query @ Wq.T ; v = value @ Wv.T          (k-projection is dead code)
    qh = q.reshape(B, H, S, DK)  (raw view: head h = a contiguous 256-row slab
                                  of q, re-chunked into rows of 64)
    kh = qh                      (repo bug: key = query.view(...))
    scores = qh @ qh^T / 8 ; mask ; softmax ; x = attn @ vh
    out = x.transpose/reshape @ Wo.T

Sharding: core c owns head h=c for both batches (2 (b,h) pairs/core).
Per pair the device computes, in score-TRANSPOSED layout S_T[j, i]:
    E_T = exp(S_T/8 - 20) * mask_T        (mask synthesized host-side; only
                                           diagonal blocks need one)
    [x_unnorm^T; l] = [vh | ones]^T-style augmented PV matmul
    po = x_unnorm^T.T @ Wo_h.T            (unnormalized partial output)
The host divides po rows by l and sums partials over heads/cores.
The causal structure of the mask (verified exactly on the host) lets the
device skip all fully-masked j-tiles. Non-causal masks fall back to numpy.
"""

import math
import sys

import numpy as np

sys.path.insert(0, "/opt/trn_rl_repo")

B, S, D, H, DK = 2, 2048, 512, 8, 64
NCORES = 8
SLAB = S // H          # 256 query rows per head-slab
CHUNK = 512            # i-chunk width (PSUM bank)
JT = 128               # j-tile height
NCHUNK = S // CHUNK    # 4
NJT = S // JT          # 16
EXP_BIAS = -20.0       # exp(s/8 - 20): overflow headroom; cancels in l-division

_cache: dict = {}


def _build_causal():
    import concourse.bass as bass
    import concourse.tile as tile
    from concourse import bacc, mybir

    f32 = mybir.dt.float32
    nc = bacc.Bacc("TRN2", target_bir_lowering=False, debug=False,
                   num_devices=NCORES)

    qT = nc.dram_tensor("qT", [B, D, SLAB], f32, kind="ExternalInput").ap()
    vT = nc.dram_tensor("vT", [B, D, SLAB], f32, kind="ExternalInput").ap()
    wqT = nc.dram_tensor("wqT", [D, D], f32, kind="ExternalInput").ap()
    wvT = nc.dram_tensor("wvT", [D, D], f32, kind="ExternalInput").ap()
    woT = nc.dram_tensor("woT", [DK, D], f32, kind="ExternalInput").ap()
    mpat = nc.dram_tensor("mpat", [4, JT, CHUNK], f32, kind="ExternalInput").ap()
    po = nc.dram_tensor("po", [B, S, D], f32, kind="ExternalOutput").ap()
    lo = nc.dram_tensor("lo", [B, 1, S], f32, kind="ExternalOutput").ap()

    KT = D // 128  # 4 k-tiles over the contraction dim of the projections

    with tile.TileContext(nc) as tc:
        with (
            tc.tile_pool(name="const", bufs=1) as constp,
            tc.tile_pool(name="acts", bufs=2) as actp,
            tc.tile_pool(name="qhT", bufs=2) as qhTp,
            tc.tile_pool(name="vh", bufs=2) as vhp,
            tc.tile_pool(name="eT", bufs=3) as eTp,
            tc.tile_pool(name="xT", bufs=2) as xTp,
            tc.tile_pool(name="fo", bufs=3) as fop,
            tc.tile_pool(name="psS", bufs=2, space="PSUM") as psS,
            tc.tile_pool(name="psX", bufs=2, space="PSUM") as psX,
            tc.tile_pool(name="psM", bufs=2, space="PSUM") as psM,
        ):
            # --- persistent constants -------------------------------------
            wq_sb, wv_sb = [], []
            for k in range(KT):
                t = constp.tile([128, D], f32, tag=f"wq{k}")
                nc.sync.dma_start(t[:], wqT[128 * k:128 * (k + 1), :])
                wq_sb.append(t)
                t = constp.tile([128, D], f32, tag=f"wv{k}")
                nc.sync.dma_start(t[:], wvT[128 * k:128 * (k + 1), :])
                wv_sb.append(t)
            # Wo^T slice duplicated into both partition halves (row-packing)
            wo_sb = constp.tile([128, D], f32, tag="wo")
            nc.sync.dma_start(wo_sb[0:64, :], woT[:, :])
            nc.sync.dma_start(wo_sb[64:128, :], woT[:, :])
            mp_sb = []
            for s_ in range(4):
                t = constp.tile([JT, CHUNK], f32, tag=f"mp{s_}")
                nc.sync.dma_start(t[:], mpat[s_, :, :])
                mp_sb.append(t)
            exp_bias = constp.tile([128, 1], f32, tag="ebias")
            nc.gpsimd.memset(exp_bias[:], EXP_BIAS)

            for bi in range(B):
                # --- load input slabs (pre-transposed on host) ------------
                qT_sb, vT_sb = [], []
                for k in range(KT):
                    t = actp.tile([128, SLAB], f32, tag=f"qt{k}")
                    nc.sync.dma_start(t[:], qT[bi, 128 * k:128 * (k + 1), :])
                    qT_sb.append(t)
                    t = actp.tile([128, SLAB], f32, tag=f"vt{k}")
                    nc.sync.dma_start(t[:], vT[bi, 128 * k:128 * (k + 1), :])
                    vT_sb.append(t)

                # --- q projection straight into qh^T [64, S] --------------
                # qhT rows 0:64 and the copy at 64:128 (for K=64 row-packing)
                qhT = qhTp.tile([128, S], f32, tag="qhT")
                qhT_v = qhT.rearrange("p (r j) -> p r j", j=H)
                for jg in range(4):
                    ps = psM.tile([128, SLAB], f32, tag="psm")
                    for k in range(KT):
                        nc.tensor.matmul(
                            ps[:], wq_sb[k][:, 128 * jg:128 * (jg + 1)],
                            qT_sb[k][:], start=(k == 0), stop=(k == KT - 1))
                    for jl in range(2):
                        jj = 2 * jg + jl
                        src = ps[64 * jl:64 * jl + 64, :]
                        nc.vector.tensor_copy(qhT_v[0:64, :, jj], src)
                        nc.vector.tensor_copy(qhT_v[64:128, :, jj], src)

                # --- v projection to natural v_slab [SLAB, D] -------------
                v_sl = []
                for rh in range(2):
                    psv = psM.tile([128, D], f32, tag="psm")
                    for k in range(KT):
                        nc.tensor.matmul(
                            psv[:], vT_sb[k][:, 128 * rh:128 * (rh + 1)],
                            wv_sb[k][:], start=(k == 0), stop=(k == KT - 1))
                    t = actp.tile([128, D], f32, tag=f"vsl{rh}")
                    nc.vector.tensor_copy(t[:], psv[:])
                    v_sl.append(t)

                # --- vh j-tiles [128, 64] + ones column (SBUF reshape DMA)
                vh = []
                for t_ in range(NJT):
                    vt = vhp.tile([128, DK + 1], f32, tag=f"vh{t_}")
                    nc.gpsimd.memset(vt[:, DK:DK + 1], 1.0)
                    r0 = 16 * (t_ % 8)
                    nc.sync.dma_start(vt[:, 0:DK],
                                      v_sl[t_ // 8][r0:r0 + 16, :])
                    vh.append(vt)

                # --- attention over i-chunks ------------------------------
                for n in range(NCHUNK):
                    n_t = 4 * n + 4  # causal: j-tiles 0 .. 4n+3
                    psx = psX.tile([DK + 1, CHUNK], f32, tag="psx")
                    for w in range(0, n_t, 2):
                        pss = psS.tile([128, 2 * CHUNK], f32, tag="pss")
                        for tw in range(2):
                            t_ = w + tw
                            p0 = 64 * tw
                            nc.tensor.matmul(
                                pss[:, CHUNK * tw:CHUNK * (tw + 1)],
                                qhT[p0:p0 + 64, JT * t_:JT * (t_ + 1)],
                                qhT[p0:p0 + 64, CHUNK * n:CHUNK * (n + 1)],
                                start=True, stop=True,
                                tile_position=(p0, 0))
                        eT = eTp.tile([128, 2 * CHUNK], f32, tag="eT")
                        nc.scalar.activation(
                            eT[:], pss[:], mybir.ActivationFunctionType.Exp,
                            bias=exp_bias[:], scale=1.0 / math.sqrt(DK))
                        for tw in range(2):
                            t_ = w + tw
                            s_ = t_ - 4 * n
                            if s_ >= 0:
                                sl = eT[:, CHUNK * tw:CHUNK * (tw + 1)]
                                nc.vector.tensor_mul(sl, sl, mp_sb[s_][:])
                        for tw in range(2):
                            t_ = w + tw
                            nc.tensor.matmul(
                                psx[:], vh[t_][:],
                                eT[:, CHUNK * tw:CHUNK * (tw + 1)],
                                start=(t_ == 0), stop=(t_ == n_t - 1),
                                skip_group_check=True)

                    # --- evacuate x^T (duplicated) and l ------------------
                    xT = xTp.tile([128, CHUNK], f32, tag="xT")
                    nc.vector.tensor_copy(xT[0:64, :], psx[0:64, :])
                    nc.vector.tensor_copy(xT[64:128, :], psx[0:64, :])
                    lsb = xTp.tile([1, CHUNK], f32, tag="lsb")
                    nc.vector.tensor_copy(lsb[:], psx[64:65, :])
                    nc.sync.dma_start(
                        lo[bi, :, CHUNK * n:CHUNK * (n + 1)], lsb[:])

                    # --- output projection (unnormalized), row-packed -----
                    for u in range(CHUNK // 128):
                        ul = u % 2
                        p0 = 64 * ul
                        psf = psM.tile([128, D], f32, tag="psm")
                        nc.tensor.matmul(
                            psf[:], xT[p0:p0 + 64, 128 * u:128 * (u + 1)],
                            wo_sb[p0:p0 + 64, :], start=True, stop=True,
                            tile_position=(p0, 0))
                        fo = fop.tile([128, D], f32, tag="fo")
                        nc.vector.tensor_copy(fo[:], psf[:])
                        r0 = CHUNK * n + 128 * u
                        nc.sync.dma_start(po[bi, r0:r0 + 128, :], fo[:])
    nc.compile()
    return nc


def _mask_patterns():
    # M_s[p, f] = 1.0 where (global j) <= (global i) for a diagonal block s
    p = np.arange(JT)[:, None]
    f = np.arange(CHUNK)[None, :]
    return np.stack(
        [(p <= f - JT * s).astype(np.float32) for s in range(4)])


def _numpy_fallback(query, key, value, mask, Wq, Wk, Wv, Wo):
    q = query @ Wq.T
    v = value @ Wv.T
    qh = q.reshape(B, H, S, DK)
    vh = v.reshape(B, H, S, DK)
    scores = np.einsum("bhqd,bhkd->bhqk", qh, qh) / math.sqrt(DK)
    scores = np.where(mask == 0, np.float32(-1e9), scores)
    scores = scores - scores.max(axis=-1, keepdims=True)
    e = np.exp(scores)
    attn = e / e.sum(axis=-1, keepdims=True)
    x = np.einsum("bhqk,bhkd->bhqd", attn, vh)
    x = x.transpose(0, 2, 1, 3).reshape(B, S, H * DK)
    return (x @ Wo.T).astype(np.float32)


def _run_device(query, value, Wq, Wv, Wo, trace=False):
    from concourse.bass_utils import run_bass_kernel_spmd

    if "nc" not in _cache:
        _cache["nc"] = _build_causal()
    nc = _cache["nc"]

    mpat = _mask_patterns()
    wqT = np.ascontiguousarray(Wq.T)
    wvT = np.ascontiguousarray(Wv.T)
    in_maps = []
    for c in range(NCORES):
        r0 = SLAB * c
        in_maps.append({
            "qT": np.ascontiguousarray(
                query[:, r0:r0 + SLAB, :].transpose(0, 2, 1)),
            "vT": np.ascontiguousarray(
                value[:, r0:r0 + SLAB, :].transpose(0, 2, 1)),
            "wqT": wqT,
            "wvT": wvT,
            "woT": np.ascontiguousarray(Wo[:, DK * c:DK * (c + 1)].T),
            "mpat": mpat,
        })
    res = run_bass_kernel_spmd(nc, in_maps, core_ids=list(range(NCORES)),
                               trace=trace)
    out = np.zeros((B, S, D), dtype=np.float64)
    for c in range(NCORES):
        pc = res.results[c]
        out += pc["po"].astype(np.float64) / \
            pc["lo"].reshape(B, S, 1).astype(np.float64)
    return out.astype(np.float32), res


_TRIL = None


def kernel(query, key, value, mask, Wq, Wk, Wv, Wo):
    global _TRIL
    query = np.asarray(query, dtype=np.float32)
    value = np.asarray(value, dtype=np.float32)
    mask = np.asarray(mask)
    Wq = np.asarray(Wq, dtype=np.float32)
    Wv = np.asarray(Wv, dtype=np.float32)
    Wo = np.asarray(Wo, dtype=np.float32)

    if _TRIL is None:
        _TRIL = np.tril(np.ones((S, S), dtype=np.int64))
    m2 = mask.reshape(S, S)
    if not np.array_equal(m2 != 0, _TRIL != 0):
        return _numpy_fallback(query, np.asarray(key), value, mask,
                               Wq, np.asarray(Wk), Wv, Wo)

    out, _ = _run_device(query, value, Wq, Wv, Wo)
    return out


# revision 7
# speedup vs baseline: 1.7094x; 1.7094x over previous
"""Multi-head attention (with the repo's k=q bug) on 8 Trainium2 NeuronCores.

Reference computation (B=2, S=2048, D=512, H=8, DK=64):
    q = query @ Wq.T ; v = value @ Wv.T          (k-projection is dead code)
    qh = q.reshape(B, H, S, DK)  (raw view: head h = a contiguous 256-row slab
                                  of q, re-chunked into rows of 64)
    kh = qh                      (repo bug: key = query.view(...))
    scores = qh @ qh^T / 8 ; mask ; softmax ; x = attn @ vh
    out = x.transpose/reshape @ Wo.T

Sharding: core c owns head h=c for both batches (2 (b,h) pairs/core).
Per pair the device computes, in score-TRANSPOSED layout S_T[j, i]:
    E_T = exp(S_T/8 - 20) * mask_T        (mask synthesized host-side; only
                                           diagonal blocks need one)
    [x_unnorm^T; l] = [vh | ones]^T-style augmented PV matmul
    po = x_unnorm^T.T @ Wo_h.T            (unnormalized partial output)
The host divides po rows by l and sums partials over heads/cores.
The causal structure of the mask (verified exactly on the host) lets the
device skip all fully-masked j-tiles. Non-causal masks fall back to numpy.
"""

import math
import sys

import numpy as np

sys.path.insert(0, "/opt/trn_rl_repo")

B, S, D, H, DK = 2, 2048, 512, 8, 64
NCORES = 8
SLAB = S // H          # 256 query rows per head-slab
CHUNK = 512            # i-chunk width (PSUM bank)
JT = 128               # j-tile height
NCHUNK = S // CHUNK    # 4
NJT = S // JT          # 16
EXP_BIAS = -20.0       # exp(s/8 - 20): overflow headroom; cancels in l-division

_cache: dict = {}


def _build_causal():
    import concourse.bass as bass
    import concourse.tile as tile
    from concourse import bacc, mybir

    f32 = mybir.dt.float32
    nc = bacc.Bacc("TRN2", target_bir_lowering=False, debug=False,
                   num_devices=NCORES)

    qT = nc.dram_tensor("qT", [B, D, SLAB], f32, kind="ExternalInput").ap()
    vT = nc.dram_tensor("vT", [B, D, SLAB], f32, kind="ExternalInput").ap()
    wqT = nc.dram_tensor("wqT", [D, D], f32, kind="ExternalInput").ap()
    wvT = nc.dram_tensor("wvT", [D, D], f32, kind="ExternalInput").ap()
    woT = nc.dram_tensor("woT", [DK, D], f32, kind="ExternalInput").ap()
    mpat = nc.dram_tensor("mpat", [4, JT, CHUNK], f32, kind="ExternalInput").ap()
    po = nc.dram_tensor("po", [B, S, D], f32, kind="ExternalOutput").ap()
    lo = nc.dram_tensor("lo", [B, 1, S], f32, kind="ExternalOutput").ap()

    KT = D // 128  # 4 k-tiles over the contraction dim of the projections

    f32r = mybir.dt.float32r

    def mm(out, lhsT, rhs, **kw):
        nc.tensor.matmul(out, lhsT.bitcast(f32r), rhs.bitcast(f32r), **kw)

    def r(ap):
        return ap.bitcast(f32r)

    with tile.TileContext(nc) as tc:
        with (
            tc.tile_pool(name="const", bufs=1) as constp,
            tc.tile_pool(name="acts", bufs=2) as actp,
            tc.tile_pool(name="qhT", bufs=2) as qhTp,
            tc.tile_pool(name="vh", bufs=2) as vhp,
            tc.tile_pool(name="eT", bufs=3) as eTp,
            tc.tile_pool(name="xT", bufs=2) as xTp,
            tc.tile_pool(name="fo", bufs=3) as fop,
            tc.tile_pool(name="psS", bufs=2, space="PSUM") as psS,
            tc.tile_pool(name="psX", bufs=2, space="PSUM") as psX,
            tc.tile_pool(name="psM", bufs=2, space="PSUM") as psM,
        ):
            # --- persistent constants -------------------------------------
            wq_sb, wv_sb = [], []
            for k in range(KT):
                t = constp.tile([128, D], f32, tag=f"wq{k}")
                nc.sync.dma_start(r(t[:]), r(wqT[128 * k:128 * (k + 1), :]))
                wq_sb.append(t)
                t = constp.tile([128, D], f32, tag=f"wv{k}")
                nc.sync.dma_start(r(t[:]), r(wvT[128 * k:128 * (k + 1), :]))
                wv_sb.append(t)
            # Wo^T slice duplicated into both partition halves (row-packing)
            wo_sb = constp.tile([128, D], f32, tag="wo")
            nc.sync.dma_start(r(wo_sb[0:64, :]), r(woT[:, :]))
            nc.sync.dma_start(r(wo_sb[64:128, :]), r(woT[:, :]))
            mp_sb = []
            for s_ in range(4):
                t = constp.tile([JT, CHUNK], f32, tag=f"mp{s_}")
                nc.sync.dma_start(t[:], mpat[s_, :, :])
                mp_sb.append(t)
            exp_bias = constp.tile([128, 1], f32, tag="ebias")
            nc.gpsimd.memset(exp_bias[:], EXP_BIAS)

            for bi in range(B):
                # --- load input slabs (pre-transposed on host) ------------
                qT_sb, vT_sb = [], []
                for k in range(KT):
                    t = actp.tile([128, SLAB], f32, tag=f"qt{k}")
                    nc.sync.dma_start(r(t[:]), r(qT[bi, 128 * k:128 * (k + 1), :]))
                    qT_sb.append(t)
                    t = actp.tile([128, SLAB], f32, tag=f"vt{k}")
                    nc.sync.dma_start(r(t[:]), r(vT[bi, 128 * k:128 * (k + 1), :]))
                    vT_sb.append(t)

                # --- q projection straight into qh^T [64, S] --------------
                # qhT rows 0:64 and the copy at 64:128 (for K=64 row-packing)
                qhT = qhTp.tile([128, S], f32, tag="qhT")
                qhT_v = qhT.rearrange("p (r j) -> p r j", j=H)
                for jg in range(4):
                    ps = psM.tile([128, SLAB], f32, tag="psm")
                    for k in range(KT):
                        mm(
                            ps[:], wq_sb[k][:, 128 * jg:128 * (jg + 1)],
                            qT_sb[k][:], start=(k == 0), stop=(k == KT - 1))
                    for jl in range(2):
                        jj = 2 * jg + jl
                        src = ps[64 * jl:64 * jl + 64, :]
                        nc.vector.tensor_copy(r(qhT_v[0:64, :, jj]), src)
                        nc.vector.tensor_copy(r(qhT_v[64:128, :, jj]), src)

                # --- v projection to natural v_slab [SLAB, D] -------------
                v_sl = []
                for rh in range(2):
                    psv = psM.tile([128, D], f32, tag="psm")
                    for k in range(KT):
                        mm(
                            psv[:], vT_sb[k][:, 128 * rh:128 * (rh + 1)],
                            wv_sb[k][:], start=(k == 0), stop=(k == KT - 1))
                    t = actp.tile([128, D], f32, tag=f"vsl{rh}")
                    nc.vector.tensor_copy(r(t[:]), psv[:])
                    v_sl.append(t)

                # --- vh j-tiles [128, 64] + ones column (SBUF reshape DMA)
                vh = []
                for t_ in range(NJT):
                    vt = vhp.tile([128, DK + 1], f32, tag=f"vh{t_}")
                    nc.gpsimd.memset(vt[:, DK:DK + 1], 1.0)
                    r0 = 16 * (t_ % 8)
                    nc.sync.dma_start(r(vt[:, 0:DK]),
                                      r(v_sl[t_ // 8][r0:r0 + 16, :]))
                    vh.append(vt)

                # --- attention over i-chunks ------------------------------
                for n in range(NCHUNK):
                    n_t = 4 * n + 4  # causal: j-tiles 0 .. 4n+3
                    psx = psX.tile([DK + 1, CHUNK], f32, tag="psx")
                    for w in range(0, n_t, 2):
                        pss = psS.tile([128, 2 * CHUNK], f32, tag="pss")
                        for tw in range(2):
                            t_ = w + tw
                            p0 = 64 * tw
                            mm(
                                pss[:, CHUNK * tw:CHUNK * (tw + 1)],
                                qhT[p0:p0 + 64, JT * t_:JT * (t_ + 1)],
                                qhT[p0:p0 + 64, CHUNK * n:CHUNK * (n + 1)],
                                start=True, stop=True,
                                tile_position=(p0, 0))
                        eT = eTp.tile([128, 2 * CHUNK], f32, tag="eT")
                        nc.scalar.activation(
                            r(eT[:]), pss[:],
                            mybir.ActivationFunctionType.Exp,
                            bias=exp_bias[:], scale=1.0 / math.sqrt(DK))
                        for tw in range(2):
                            t_ = w + tw
                            s_ = t_ - 4 * n
                            if s_ >= 0:
                                sl = r(eT[:, CHUNK * tw:CHUNK * (tw + 1)])
                                nc.vector.tensor_mul(sl, sl, r(mp_sb[s_][:]))
                        for tw in range(2):
                            t_ = w + tw
                            mm(
                                psx[:], vh[t_][:],
                                eT[:, CHUNK * tw:CHUNK * (tw + 1)],
                                start=(t_ == 0), stop=(t_ == n_t - 1),
                                skip_group_check=True)

                    # --- evacuate x^T (duplicated) and l ------------------
                    xT = xTp.tile([128, CHUNK], f32, tag="xT")
                    nc.vector.tensor_copy(r(xT[0:64, :]), psx[0:64, :])
                    nc.vector.tensor_copy(r(xT[64:128, :]), psx[0:64, :])
                    lsb = xTp.tile([1, CHUNK], f32, tag="lsb")
                    nc.vector.tensor_copy(lsb[:], psx[64:65, :])
                    nc.sync.dma_start(
                        lo[bi, :, CHUNK * n:CHUNK * (n + 1)], lsb[:])

                    # --- output projection (unnormalized), row-packed -----
                    for u in range(CHUNK // 128):
                        ul = u % 2
                        p0 = 64 * ul
                        psf = psM.tile([128, D], f32, tag="psm")
                        mm(
                            psf[:], xT[p0:p0 + 64, 128 * u:128 * (u + 1)],
                            wo_sb[p0:p0 + 64, :], start=True, stop=True,
                            tile_position=(p0, 0))
                        fo = fop.tile([128, D], f32, tag="fo")
                        nc.vector.tensor_copy(fo[:], psf[:])
                        r0 = CHUNK * n + 128 * u
                        nc.sync.dma_start(po[bi, r0:r0 + 128, :], fo[:])
    nc.compile()
    return nc


def _mask_patterns():
    # M_s[p, f] = 1.0 where (global j) <= (global i) for a diagonal block s
    p = np.arange(JT)[:, None]
    f = np.arange(CHUNK)[None, :]
    return np.stack(
        [(p <= f - JT * s).astype(np.float32) for s in range(4)])


def _numpy_fallback(query, key, value, mask, Wq, Wk, Wv, Wo):
    q = query @ Wq.T
    v = value @ Wv.T
    qh = q.reshape(B, H, S, DK)
    vh = v.reshape(B, H, S, DK)
    scores = np.einsum("bhqd,bhkd->bhqk", qh, qh) / math.sqrt(DK)
    scores = np.where(mask == 0, np.float32(-1e9), scores)
    scores = scores - scores.max(axis=-1, keepdims=True)
    e = np.exp(scores)
    attn = e / e.sum(axis=-1, keepdims=True)
    x = np.einsum("bhqk,bhkd->bhqd", attn, vh)
    x = x.transpose(0, 2, 1, 3).reshape(B, S, H * DK)
    return (x @ Wo.T).astype(np.float32)


def _run_device(query, value, Wq, Wv, Wo, trace=False):
    from concourse.bass_utils import run_bass_kernel_spmd

    if "nc" not in _cache:
        _cache["nc"] = _build_causal()
    nc = _cache["nc"]

    mpat = _mask_patterns()
    wqT = np.ascontiguousarray(Wq.T)
    wvT = np.ascontiguousarray(Wv.T)
    in_maps = []
    for c in range(NCORES):
        r0 = SLAB * c
        in_maps.append({
            "qT": np.ascontiguousarray(
                query[:, r0:r0 + SLAB, :].transpose(0, 2, 1)),
            "vT": np.ascontiguousarray(
                value[:, r0:r0 + SLAB, :].transpose(0, 2, 1)),
            "wqT": wqT,
            "wvT": wvT,
            "woT": np.ascontiguousarray(Wo[:, DK * c:DK * (c + 1)].T),
            "mpat": mpat,
        })
    res = run_bass_kernel_spmd(nc, in_maps, core_ids=list(range(NCORES)),
                               trace=trace)
    out = np.zeros((B, S, D), dtype=np.float64)
    for c in range(NCORES):
        pc = res.results[c]
        out += pc["po"].astype(np.float64) / \
            pc["lo"].reshape(B, S, 1).astype(np.float64)
    return out.astype(np.float32), res


_TRIL = None


def kernel(query, key, value, mask, Wq, Wk, Wv, Wo):
    global _TRIL
    query = np.asarray(query, dtype=np.float32)
    value = np.asarray(value, dtype=np.float32)
    mask = np.asarray(mask)
    Wq = np.asarray(Wq, dtype=np.float32)
    Wv = np.asarray(Wv, dtype=np.float32)
    Wo = np.asarray(Wo, dtype=np.float32)

    if _TRIL is None:
        _TRIL = np.tril(np.ones((S, S), dtype=np.int64))
    m2 = mask.reshape(S, S)
    if not np.array_equal(m2 != 0, _TRIL != 0):
        return _numpy_fallback(query, np.asarray(key), value, mask,
                               Wq, np.asarray(Wk), Wv, Wo)

    out, _ = _run_device(query, value, Wq, Wv, Wo)
    return out


# revision 8
# speedup vs baseline: 1.8171x; 1.0630x over previous
"""Multi-head attention (with the repo's k=q bug) on 8 Trainium2 NeuronCores.

Reference computation (B=2, S=2048, D=512, H=8, DK=64):
    q = query @ Wq.T ; v = value @ Wv.T          (k-projection is dead code)
    qh = q.reshape(B, H, S, DK)  (raw view: head h = a contiguous 256-row slab
                                  of q, re-chunked into rows of 64)
    kh = qh                      (repo bug: key = query.view(...))
    scores = qh @ qh^T / 8 ; mask ; softmax ; x = attn @ vh
    out = x.transpose/reshape @ Wo.T

Sharding: core c owns head h=c for both batches (2 (b,h) pairs/core).
Per pair the device computes, in score-TRANSPOSED layout S_T[j, i]:
    E_T = exp(S_T/8 - 20) * mask_T        (mask synthesized host-side; only
                                           diagonal blocks need one)
    [x_unnorm^T; l] = [vh | ones]^T-style augmented PV matmul
    po = x_unnorm^T.T @ Wo_h.T            (unnormalized partial output)
The host divides po rows by l and sums partials over heads/cores.
The causal structure of the mask (verified exactly on the host) lets the
device skip all fully-masked j-tiles. Non-causal masks fall back to numpy.
"""

import math
import sys

import numpy as np

sys.path.insert(0, "/opt/trn_rl_repo")

B, S, D, H, DK = 2, 2048, 512, 8, 64
NCORES = 8
SLAB = S // H          # 256 query rows per head-slab
CHUNK = 512            # i-chunk width (PSUM bank)
JT = 128               # j-tile height
NCHUNK = S // CHUNK    # 4
NJT = S // JT          # 16
EXP_BIAS = -20.0       # exp(s/8 - 20): overflow headroom; cancels in l-division

_cache: dict = {}


def _build_causal():
    import concourse.bass as bass
    import concourse.tile as tile
    from concourse import bacc, mybir

    f32 = mybir.dt.float32
    nc = bacc.Bacc("TRN2", target_bir_lowering=False, debug=False,
                   num_devices=NCORES)

    qT = nc.dram_tensor("qT", [B, D, SLAB], f32, kind="ExternalInput").ap()
    vT = nc.dram_tensor("vT", [B, D, SLAB], f32, kind="ExternalInput").ap()
    wqT = nc.dram_tensor("wqT", [D, D], f32, kind="ExternalInput").ap()
    wvT = nc.dram_tensor("wvT", [D, D], f32, kind="ExternalInput").ap()
    woT = nc.dram_tensor("woT", [DK, D], f32, kind="ExternalInput").ap()
    mpat = nc.dram_tensor("mpat", [4, JT, CHUNK], mybir.dt.bfloat16,
                          kind="ExternalInput").ap()
    po = nc.dram_tensor("po", [B, S, D], f32, kind="ExternalOutput").ap()
    lo = nc.dram_tensor("lo", [B, 1, S], f32, kind="ExternalOutput").ap()

    KT = D // 128  # 4 k-tiles over the contraction dim of the projections

    f32r = mybir.dt.float32r
    bf16 = mybir.dt.bfloat16

    def mm(out, lhsT, rhs, **kw):
        nc.tensor.matmul(out, lhsT.bitcast(f32r), rhs.bitcast(f32r), **kw)

    def r(ap):
        return ap.bitcast(f32r)

    with tile.TileContext(nc) as tc:
        with (
            tc.tile_pool(name="const", bufs=1) as constp,
            tc.tile_pool(name="acts", bufs=2) as actp,
            tc.tile_pool(name="qhT", bufs=2) as qhTp,
            tc.tile_pool(name="vh", bufs=2) as vhp,
            tc.tile_pool(name="eT", bufs=3) as eTp,
            tc.tile_pool(name="xT", bufs=2) as xTp,
            tc.tile_pool(name="fo", bufs=3) as fop,
            tc.tile_pool(name="psS", bufs=2, space="PSUM") as psS,
            tc.tile_pool(name="psX", bufs=2, space="PSUM") as psX,
            tc.tile_pool(name="psM", bufs=2, space="PSUM") as psM,
        ):
            # --- persistent constants -------------------------------------
            wq_sb, wv_sb = [], []
            for k in range(KT):
                t = constp.tile([128, D], f32, tag=f"wq{k}")
                nc.sync.dma_start(r(t[:]), r(wqT[128 * k:128 * (k + 1), :]))
                wq_sb.append(t)
                t = constp.tile([128, D], f32, tag=f"wv{k}")
                nc.sync.dma_start(r(t[:]), r(wvT[128 * k:128 * (k + 1), :]))
                wv_sb.append(t)
            # Wo^T slice duplicated into both partition halves (row-packing)
            wo_sb = constp.tile([128, D], f32, tag="wo")
            nc.sync.dma_start(r(wo_sb[0:64, :]), r(woT[:, :]))
            nc.sync.dma_start(r(wo_sb[64:128, :]), r(woT[:, :]))
            mp_sb = []
            for s_ in range(4):
                t = constp.tile([JT, CHUNK], bf16, tag=f"mp{s_}")
                nc.gpsimd.dma_start(t[:], mpat[s_, :, :])
                mp_sb.append(t)
            exp_bias = constp.tile([128, 1], f32, tag="ebias")
            nc.gpsimd.memset(exp_bias[:], EXP_BIAS)

            for bi in range(B):
                # --- load input slabs (pre-transposed on host) ------------
                qT_sb, vT_sb = [], []
                for k in range(KT):
                    t = actp.tile([128, SLAB], f32, tag=f"qt{k}")
                    nc.sync.dma_start(r(t[:]), r(qT[bi, 128 * k:128 * (k + 1), :]))
                    qT_sb.append(t)
                    t = actp.tile([128, SLAB], f32, tag=f"vt{k}")
                    nc.sync.dma_start(r(t[:]), r(vT[bi, 128 * k:128 * (k + 1), :]))
                    vT_sb.append(t)

                # --- q projection straight into qh^T [64, S] --------------
                # qhT rows 0:64 and the copy at 64:128 (for K=64 row-packing)
                qhT = qhTp.tile([128, S], f32, tag="qhT")
                qhT_v = qhT.rearrange("p (r j) -> p r j", j=H)
                for jg in range(4):
                    ps = psM.tile([128, SLAB], f32, tag="psm")
                    for k in range(KT):
                        mm(
                            ps[:], wq_sb[k][:, 128 * jg:128 * (jg + 1)],
                            qT_sb[k][:], start=(k == 0), stop=(k == KT - 1))
                    for jl in range(2):
                        jj = 2 * jg + jl
                        src = ps[64 * jl:64 * jl + 64, :]
                        nc.vector.tensor_copy(r(qhT_v[0:64, :, jj]), src)
                        nc.vector.tensor_copy(r(qhT_v[64:128, :, jj]), src)

                # --- v projection to natural v_slab [SLAB, D] -------------
                v_sl = []
                for rh in range(2):
                    psv = psM.tile([128, D], f32, tag="psm")
                    for k in range(KT):
                        mm(
                            psv[:], vT_sb[k][:, 128 * rh:128 * (rh + 1)],
                            wv_sb[k][:], start=(k == 0), stop=(k == KT - 1))
                    t = actp.tile([128, D], bf16, tag=f"vsl{rh}")
                    nc.vector.tensor_copy(t[:], psv[:])
                    v_sl.append(t)

                # --- vh j-tiles [128, 64] + ones column (SBUF reshape DMA)
                vh = []
                for t_ in range(NJT):
                    vt = vhp.tile([128, DK + 1], bf16, tag=f"vh{t_}")
                    nc.gpsimd.memset(vt[:, DK:DK + 1], 1.0)
                    r0 = 16 * (t_ % 8)
                    nc.gpsimd.dma_start(vt[:, 0:DK],
                                        v_sl[t_ // 8][r0:r0 + 16, :])
                    vh.append(vt)

                # --- attention over i-chunks ------------------------------
                for n in range(NCHUNK):
                    n_t = 4 * n + 4  # causal: j-tiles 0 .. 4n+3
                    psx = psX.tile([DK + 1, CHUNK], f32, tag="psx")
                    for w in range(0, n_t, 2):
                        pss = psS.tile([128, 2 * CHUNK], f32, tag="pss")
                        for tw in range(2):
                            t_ = w + tw
                            p0 = 64 * tw
                            mm(
                                pss[:, CHUNK * tw:CHUNK * (tw + 1)],
                                qhT[p0:p0 + 64, JT * t_:JT * (t_ + 1)],
                                qhT[p0:p0 + 64, CHUNK * n:CHUNK * (n + 1)],
                                start=True, stop=True,
                                tile_position=(p0, 0))
                        eT = eTp.tile([128, 2 * CHUNK], bf16, tag="eT")
                        nc.scalar.activation(
                            eT[:], pss[:],
                            mybir.ActivationFunctionType.Exp,
                            bias=exp_bias[:], scale=1.0 / math.sqrt(DK))
                        for tw in range(2):
                            t_ = w + tw
                            s_ = t_ - 4 * n
                            if s_ >= 0:
                                sl = eT[:, CHUNK * tw:CHUNK * (tw + 1)]
                                nc.vector.tensor_mul(sl, sl, mp_sb[s_][:])
                        for tw in range(2):
                            t_ = w + tw
                            nc.tensor.matmul(
                                psx[:], vh[t_][:],
                                eT[:, CHUNK * tw:CHUNK * (tw + 1)],
                                start=(t_ == 0), stop=(t_ == n_t - 1),
                                skip_group_check=True)

                    # --- evacuate x^T (duplicated) and l ------------------
                    xT = xTp.tile([128, CHUNK], f32, tag="xT")
                    nc.vector.tensor_copy(r(xT[0:64, :]), psx[0:64, :])
                    nc.vector.tensor_copy(r(xT[64:128, :]), psx[0:64, :])
                    lsb = xTp.tile([1, CHUNK], f32, tag="lsb")
                    nc.scalar.copy(lsb[:], psx[64:65, :])
                    nc.gpsimd.dma_start(
                        lo[bi, :, CHUNK * n:CHUNK * (n + 1)], lsb[:])

                    # --- output projection (unnormalized), row-packed -----
                    for u in range(CHUNK // 128):
                        ul = u % 2
                        p0 = 64 * ul
                        psf = psM.tile([128, D], f32, tag="psm")
                        mm(
                            psf[:], xT[p0:p0 + 64, 128 * u:128 * (u + 1)],
                            wo_sb[p0:p0 + 64, :], start=True, stop=True,
                            tile_position=(p0, 0))
                        fo = fop.tile([128, D], f32, tag="fo")
                        if u % 2 == 0:
                            nc.vector.tensor_copy(fo[:], psf[:])
                        else:
                            nc.scalar.copy(fo[:], psf[:])
                        r0 = CHUNK * n + 128 * u
                        nc.sync.dma_start(po[bi, r0:r0 + 128, :], fo[:])
    nc.compile()
    return nc


def _mask_patterns():
    # M_s[p, f] = 1.0 where (global j) <= (global i) for a diagonal block s
    p = np.arange(JT)[:, None]
    f = np.arange(CHUNK)[None, :]
    import ml_dtypes
    return np.stack(
        [(p <= f - JT * s) for s in range(4)]).astype(ml_dtypes.bfloat16)


def _numpy_fallback(query, key, value, mask, Wq, Wk, Wv, Wo):
    q = query @ Wq.T
    v = value @ Wv.T
    qh = q.reshape(B, H, S, DK)
    vh = v.reshape(B, H, S, DK)
    scores = np.einsum("bhqd,bhkd->bhqk", qh, qh) / math.sqrt(DK)
    scores = np.where(mask == 0, np.float32(-1e9), scores)
    scores = scores - scores.max(axis=-1, keepdims=True)
    e = np.exp(scores)
    attn = e / e.sum(axis=-1, keepdims=True)
    x = np.einsum("bhqk,bhkd->bhqd", attn, vh)
    x = x.transpose(0, 2, 1, 3).reshape(B, S, H * DK)
    return (x @ Wo.T).astype(np.float32)


def _run_device(query, value, Wq, Wv, Wo, trace=False):
    from concourse.bass_utils import run_bass_kernel_spmd

    if "nc" not in _cache:
        _cache["nc"] = _build_causal()
    nc = _cache["nc"]

    mpat = _mask_patterns()
    wqT = np.ascontiguousarray(Wq.T)
    wvT = np.ascontiguousarray(Wv.T)
    in_maps = []
    for c in range(NCORES):
        r0 = SLAB * c
        in_maps.append({
            "qT": np.ascontiguousarray(
                query[:, r0:r0 + SLAB, :].transpose(0, 2, 1)),
            "vT": np.ascontiguousarray(
                value[:, r0:r0 + SLAB, :].transpose(0, 2, 1)),
            "wqT": wqT,
            "wvT": wvT,
            "woT": np.ascontiguousarray(Wo[:, DK * c:DK * (c + 1)].T),
            "mpat": mpat,
        })
    res = run_bass_kernel_spmd(nc, in_maps, core_ids=list(range(NCORES)),
                               trace=trace)
    out = np.zeros((B, S, D), dtype=np.float64)
    for c in range(NCORES):
        pc = res.results[c]
        out += pc["po"].astype(np.float64) / \
            pc["lo"].reshape(B, S, 1).astype(np.float64)
    return out.astype(np.float32), res


_TRIL = None


def kernel(query, key, value, mask, Wq, Wk, Wv, Wo):
    global _TRIL
    query = np.asarray(query, dtype=np.float32)
    value = np.asarray(value, dtype=np.float32)
    mask = np.asarray(mask)
    Wq = np.asarray(Wq, dtype=np.float32)
    Wv = np.asarray(Wv, dtype=np.float32)
    Wo = np.asarray(Wo, dtype=np.float32)

    if _TRIL is None:
        _TRIL = np.tril(np.ones((S, S), dtype=np.int64))
    m2 = mask.reshape(S, S)
    if not np.array_equal(m2 != 0, _TRIL != 0):
        return _numpy_fallback(query, np.asarray(key), value, mask,
                               Wq, np.asarray(Wk), Wv, Wo)

    out, _ = _run_device(query, value, Wq, Wv, Wo)
    return out
